# revision 1
# baseline (speedup 1.0000x reference)
"""Causal MHA (B=2, T=2048, D=1024, H=16, HD=64) on 8 TRN2 NeuronCores.

Sharding (Megatron-style, per the hint): core c = b*4 + hg handles batch b
and head group hg (4 heads). Each core computes
  qkv shard -> 4-head causal attention -> partial proj output [T, D]
and the host reduces the 4 per-batch partials and adds bproj.

Device dataflow (zero on-device transposes; host pre-transposes weights/x):
  xT [D, T] (host: x[b].T)
  qT,kT = Wqk_shard @ x  computed as matmul(lhsT=wqkT, rhs=xT) -> [r, t] layout
  v     = x @ Wv_shard.T computed as matmul(lhsT=xT, rhs=wvT)  -> [t, r] layout
  S^T[j, i] = matmul(lhsT=kT, rhs=qT)  (K=64 contraction, 2 heads row-packed
              via base_partition 0/64)
  expS = exp(0.125 * S^T)  (no max-subtraction: logits are O(1) by construction)
  causal handled structurally; diagonal 128x128 sub-blocks masked
          multiplicatively post-exp
  scores^T accumulated per head pair into one PSUM bank (f32r matmuls must
          write from partition base 0, so head B's lhsT is zero-padded by 65
          columns to land at rows 65:128; head A carries a ones column whose
          output row 64 is the softmax denominator; B's e=63 column and
          denominator ride a second [j,33] matmul into a d bank)
  normalize via ones-outer-product broadcast matmuls + reciprocal_approx_fast,
          then head B's rows are partition-shifted into the pair-stacked
          scores^T layout with two small sbuf->sbuf DMAs
  out = matmul(lhsT=scores^T, rhs=wprojT) -> [t, j] natural layout

Scheduling: QKV t-chunks and attention i-chunks are interleaved causally
(attention for i-chunk c needs only chunks <= c), and the next chunk's QKV
groups plus the previous chunk's proj units are manually interleaved between
attention jt iterations as PE filler for the ACT-bound exp pipeline.

All matmul inputs are float32r (TF32-like single-pass PE rate, ~1.5e-4 rel
err); accumulation is fp32 in PSUM. The m==3 diagonal tile is widened to 256
columns (zeros||tri mask) to avoid the f32r N<256 4-cycle/row penalty.
"""

import numpy as np

import concourse.mybir as mybir
import concourse.tile as tile
from concourse import bacc
from concourse.bass_utils import run_bass_kernel_spmd
from concourse.masks import make_upper_triangular

B, T, D, H = 2, 2048, 1024, 16
HD = 64
NCORES = 8
HG = 4          # head groups (cores per batch)
HPG = H // HG   # heads per core = 4
P = 128
F32 = mybir.dt.float32
F32R = mybir.dt.float32r
EXP = mybir.ActivationFunctionType.Exp

# module-level knobs for test harness
TRACE = False
LAST_RESULTS = None

_cached_nc = None


def _build_nc():
    nc = bacc.Bacc("TRN2", target_bir_lowering=False, debug=False)

    xT_d = nc.dram_tensor("xT", [D, T], F32R, kind="ExternalInput")
    wqkT_d = nc.dram_tensor("wqkT", [D, 512], F32R, kind="ExternalInput")
    wvT_d = nc.dram_tensor("wvT", [D, 256], F32R, kind="ExternalInput")
    bqk_d = nc.dram_tensor("bqk", [P, 4], F32, kind="ExternalInput")
    bv_d = nc.dram_tensor("bv", [256], F32, kind="ExternalInput")
    wpT_d = nc.dram_tensor("wpT", [256, D], F32R, kind="ExternalInput")
    out_d = nc.dram_tensor("out", [T, D], F32, kind="ExternalOutput")

    import concourse.bass as bass

    with tile.TileContext(nc) as tc:
        with (
            tc.tile_pool(name="consts", bufs=1) as consts,
            tc.tile_pool(name="work", bufs=2) as work,
            tc.tile_pool(name="es", bufs=6) as es_pool,
            tc.tile_pool(name="norm", bufs=3) as norm,
            tc.tile_pool(name="outp", bufs=4) as outp,
            tc.tile_pool(name="psA", bufs=2, space="PSUM") as psA,
            tc.tile_pool(name="psQ", bufs=2, space="PSUM") as psQ,
            tc.tile_pool(name="psO", bufs=1, space="PSUM") as psO,
        ):
            # ---- persistent tiles (loads emitted in consumption order) ----
            wqkT_sb = consts.tile([P, 8, 512], F32R)
            wqkT_r = wqkT_d[:].rearrange("(ko p) r -> p ko r", p=P)
            wvT_sb = consts.tile([P, 8, 256], F32R)
            wpT_sb = consts.tile([P, 2, D], F32R)
            bqk_sb = consts.tile([P, 4], F32)
            nc.sync.dma_start(bqk_sb[:], bqk_d[:])
            bv_sb = consts.tile([P, 256], F32)
            nc.sync.dma_start(
                bv_sb[:],
                bass.AP(tensor=bv_d, offset=0, ap=[[0, P], [1, 256]]),
            )
            mask_sb = consts.tile([P, P], F32)
            make_upper_triangular(nc, mask_sb[:], val=1.0, diag=True)
            # zeros||tri mask for the m==3 diagonal tile, which is widened to
            # 256 cols to dodge the f32r narrow-matmul (N<256 -> 4 cyc/row)
            # penalty; cols [0:128] are fully above the diagonal.
            mask256 = consts.tile([P, 256], F32)
            nc.vector.memset(mask256[:, 0:128], 0.0)
            nc.vector.tensor_copy(mask256[:, 128:256], mask_sb[:])
            ones_f = consts.tile([P, 64], F32)
            nc.vector.memset(ones_f[:], 1.0)
            zeros_f = consts.tile([P, 64], F32)
            nc.vector.memset(zeros_f[:], 0.0)
            # bcolB[p, 0] = 1, [1:65] = 0, [65:128] = 1 (bc lhsT for head B:
            # broadcasts 1/denB to rows {0, 65:128})
            bcolB = consts.tile([P, 128], F32R)
            nc.vector.tensor_copy(bcolB[:, 0:1], ones_f[:, 0:1])
            nc.vector.tensor_copy(bcolB[:, 1:65], zeros_f[:])
            nc.vector.tensor_copy(bcolB[:, 65:128], ones_f[:, 0:63])
            ones65_r = consts.tile([P, 65], F32R)
            nc.vector.tensor_copy(ones65_r[:], ones_f[:, 0:1].to_broadcast([P, 65]))

            qT_sb = consts.tile([P, 2, T], F32R)
            kT_sb = consts.tile([P, 2, T], F32R)
            # per pair: head A's v + ones col (M=65: scores rows 0:64 and
            # denominator row 64); head B's v[e 0:62] zero-padded by 65 so its
            # AV output lands at PSUM rows 65:128 with dst base 0; B's e=63
            # column and ones go to the d bank (rows 0 / 32) via vaB63.
            vaA_sb = consts.tile([P, 16, 2, 65], F32R)
            nc.vector.tensor_copy(
                vaA_sb[:, :, :, 64:65],
                ones_f[:, None, None, 0:1].to_broadcast([P, 16, 2, 1]),
            )
            vaB_sb = consts.tile([P, 16, 2, 128], F32R)
            nc.vector.tensor_copy(
                vaB_sb[:, :, :, 0:65],
                zeros_f[:, None, None, 0:1].to_broadcast([P, 16, 2, 65]),
            )
            vaB63_sb = consts.tile([P, 16, 2, 33], F32R)
            nc.vector.tensor_copy(
                vaB63_sb[:, :, :, 1:32],
                zeros_f[:, None, None, 0:31].to_broadcast([P, 16, 2, 31]),
            )
            nc.vector.tensor_copy(
                vaB63_sb[:, :, :, 32:33],
                ones_f[:, None, None, 0:1].to_broadcast([P, 16, 2, 1]),
            )
            scT_sb = consts.tile([P, 2, T], F32R)

            xT_r = xT_d[:].rearrange("(ko p) t -> p ko t", p=P)

            # chunk-0 x interleaved with wqkT so the first q/k matmul can
            # start after ~512KB of traffic; bulkier weights follow.
            xt0 = work.tile([P, 8, 512], F32R, tag="xt")
            for ko in range(8):
                nc.sync.dma_start(xt0[:, ko, :], xT_r[:, ko, 0:512])
                nc.sync.dma_start(wqkT_sb[:, ko, :], wqkT_r[:, ko, :])
            nc.sync.dma_start(
                wvT_sb[:], wvT_d[:].rearrange("(ko p) r -> p ko r", p=P)
            )
            nc.sync.dma_start(
                wpT_sb[:], wpT_d[:].rearrange("(co p) j -> p co j", p=P)
            )

            # ---- work-unit emitters ----
            def xt_dma(tch):
                xt = work.tile([P, 8, 512], F32R, tag="xt")
                for ko in range(8):
                    nc.sync.dma_start(
                        xt[:, ko, :],
                        xT_r[:, ko, tch * 512:(tch + 1) * 512],
                    )
                return xt

            def qkv_group(xt, tch, g):
                """g 0..3: q/k r-tiles; g 4..7: v t-tiles."""
                if g < 4:
                    rt = g
                    ps = psQ.tile([P, 512], F32, tag="qkv")
                    for ko in range(8):
                        nc.tensor.matmul(
                            ps[:],
                            wqkT_sb[:, ko, rt * 128:(rt + 1) * 128],
                            xt[:, ko, :],
                            start=(ko == 0),
                            stop=(ko == 7),
                        )
                    dest = (qT_sb if rt < 2 else kT_sb)[
                        :, rt % 2, tch * 512:(tch + 1) * 512
                    ]
                    nc.vector.tensor_scalar_add(
                        out=dest, in0=ps[:], scalar1=bqk_sb[:, rt:rt + 1]
                    )
                else:
                    tt4 = g - 4
                    tg = tch * 4 + tt4
                    psv = psQ.tile([P, 256], F32, tag="qkv")
                    for ko in range(8):
                        nc.tensor.matmul(
                            psv[:],
                            xt[:, ko, tt4 * 128:(tt4 + 1) * 128],
                            wvT_sb[:, ko, :],
                            start=(ko == 0),
                            stop=(ko == 7),
                        )
                    psv_h = psv[:].rearrange("p (pr h e) -> p pr h e", pr=2, h=2)
                    bv_h = bv_sb[:].rearrange("p (pr h e) -> p pr h e", pr=2, h=2)
                    nc.vector.tensor_tensor(
                        out=vaA_sb[:, tg, :, 0:64],
                        in0=psv_h[:, :, 0, :],
                        in1=bv_h[:, :, 0, :],
                        op=mybir.AluOpType.add,
                    )
                    nc.vector.tensor_tensor(
                        out=vaB_sb[:, tg, :, 65:128],
                        in0=psv_h[:, :, 1, 0:63],
                        in1=bv_h[:, :, 1, 0:63],
                        op=mybir.AluOpType.add,
                    )
                    nc.vector.tensor_tensor(
                        out=vaB63_sb[:, tg, :, 0:1],
                        in0=psv_h[:, :, 1, 63:64],
                        in1=bv_h[:, :, 1, 63:64],
                        op=mybir.AluOpType.add,
                    )

            def proj_unit(ic, u):
                """u 0..7: tt4 = u // 2, jc = u % 2."""
                tt = ic * 4 + u // 2
                jc = u % 2
                pp = psQ.tile([P, 512], F32, tag="qkv")
                for pr in range(2):
                    nc.tensor.matmul(
                        pp[:],
                        scT_sb[:, pr, tt * 128:(tt + 1) * 128],
                        wpT_sb[:, pr, jc * 512:(jc + 1) * 512],
                        start=(pr == 0),
                        stop=(pr == 1),
                    )
                o_sb = outp.tile([P, 512], F32, tag="out")
                nc.vector.tensor_copy(o_sb[:], pp[:])
                nc.sync.dma_start(
                    out_d[tt * 128:(tt + 1) * 128, jc * 512:(jc + 1) * 512],
                    o_sb[:],
                )

            def attention_ic(ic, pending):
                """pending: list of zero-arg emitters interleaved between
                jt iterations (PE filler for the ACT-bound exp pipeline)."""
                niter = 2 * (4 * ic + 4)
                # reserve up to 2 fillers per pair-normalize boundary; spread
                # the rest across jt iterations (PE filler for the ACT-bound
                # exp pipeline)
                if ic == 3:
                    nres = min(6, len(pending))
                    nb0 = 2
                elif len(pending) > 4:
                    nres = 4
                    nb0 = 2
                else:
                    nres = 0
                    nb0 = 0
                boundary = [pending[:nb0], pending[nb0:nres]]
                rest = pending[nres:]
                k = len(rest)
                sched = {}
                for idx in range(k):
                    sched.setdefault(idx * niter // k, []).append(rest[idx])
                it = 0
                for pr in range(2):
                    o_ps = psO.tile([P, 512], F32, tag="o")
                    d_ps = psO.tile([P, 512], F32, tag="d")
                    njt = 4 * ic + 4
                    for jt in range(njt):
                        m = jt - 4 * ic
                        i_lo = 0 if m <= 0 else (256 if m == 3 else 128 * m)
                        s_ps = psA.tile([P, 1024], F32, tag="s")
                        for h in range(2):
                            nc.tensor.matmul(
                                s_ps[:, h * 512 + i_lo:(h + 1) * 512],
                                kT_sb[64 * h:64 * h + 64, pr,
                                      jt * 128:(jt + 1) * 128],
                                qT_sb[64 * h:64 * h + 64, pr,
                                      ic * 512 + i_lo:(ic + 1) * 512],
                                start=True,
                                stop=True,
                            )
                        es = es_pool.tile([P, 2, 512], F32R, tag="es")
                        s_view = s_ps[:].rearrange("p (h i) -> p h i", h=2)
                        nc.scalar.activation(
                            out=es[:, :, i_lo:],
                            in_=s_view[:, :, i_lo:],
                            func=EXP,
                            scale=0.125,
                        )
                        if m == 3:
                            nc.vector.tensor_tensor(
                                out=es[:, :, 256:512],
                                in0=es[:, :, 256:512],
                                in1=mask256[:, None, :].to_broadcast(
                                    [P, 2, 256]
                                ),
                                op=mybir.AluOpType.mult,
                            )
                        elif m >= 0:
                            nc.vector.tensor_tensor(
                                out=es[:, :, i_lo:i_lo + 128],
                                in0=es[:, :, i_lo:i_lo + 128],
                                in1=mask_sb[:, None, :].to_broadcast([P, 2, P]),
                                op=mybir.AluOpType.mult,
                            )
                        # head B first at jt==0: its start covers rows 0:128
                        nc.tensor.matmul(
                            o_ps[0:128, i_lo:],
                            vaB_sb[:, jt, pr, :],
                            es[:, 1, i_lo:],
                            start=(jt == 0),
                            stop=(jt == njt - 1),
                            skip_group_check=True,
                        )
                        nc.tensor.matmul(
                            o_ps[0:65, i_lo:],
                            vaA_sb[:, jt, pr, :],
                            es[:, 0, i_lo:],
                            start=(jt == 0),
                            stop=(jt == njt - 1),
                            skip_group_check=True,
                        )
                        nc.tensor.matmul(
                            d_ps[0:33, i_lo:],
                            vaB63_sb[:, jt, pr, :],
                            es[:, 1, i_lo:],
                            start=(jt == 0),
                            stop=(jt == njt - 1),
                            skip_group_check=True,
                        )
                        for fill in sched.get(it, ()):
                            fill()
                        it += 1
                    # normalize -> scores^T (pair-stacked rows)
                    # denA sits at o_ps row 64 (ones col of vaA); denB at
                    # d_ps row 32; B's e=63 scores at d_ps row 0.
                    for fill in boundary[pr]:
                        fill()
                    den = norm.tile([P, 1024], F32R, tag="den")
                    nc.scalar.copy(out=den[64:65, 0:512], in_=o_ps[64:65, :])
                    nc.vector.tensor_copy(den[0:33, 512:1024], d_ps[0:33, :])
                    bcp = psA.tile([P, 1024], F32, tag="s")
                    nc.tensor.matmul(
                        bcp[0:65, 0:512],
                        ones65_r[64:65, :],
                        den[64:65, 0:512],
                        start=True, stop=True,
                    )
                    nc.tensor.matmul(
                        bcp[0:128, 512:1024],
                        bcolB[32:33, :],
                        den[32:33, 512:1024],
                        start=True, stop=True,
                    )
                    bc_sb = norm.tile([P, 1024], F32, tag="bc")
                    nc.vector.reciprocal_approx_fast(
                        out=bc_sb[:, :], in_=bcp[:, :]
                    )
                    # head B first: its partition-shift DMAs are on the
                    # critical path into proj. e 0:62 at o rows 65:128, e 63
                    # at d row 0 -- normalize into tmpB, then shift via DMA.
                    tmpB = norm.tile([P, 512], F32R, tag="tmpB")
                    # start at 64 (engine APs need 32-aligned partition base);
                    # row 64 computes junk (denA * recip(0)) and is never read.
                    nc.vector.tensor_tensor(
                        out=tmpB[64:128, :],
                        in0=o_ps[64:128, :],
                        in1=bc_sb[64:128, 512:1024],
                        op=mybir.AluOpType.mult,
                    )
                    nc.sync.dma_start(
                        scT_sb[64:127, pr, ic * 512:(ic + 1) * 512],
                        tmpB[65:128, :],
                    )
                    nc.vector.tensor_tensor(
                        out=tmpB[0:1, :],
                        in0=d_ps[0:1, :],
                        in1=bc_sb[0:1, 512:1024],
                        op=mybir.AluOpType.mult,
                    )
                    nc.sync.dma_start(
                        scT_sb[127:128, pr, ic * 512:(ic + 1) * 512],
                        tmpB[0:1, :],
                    )
                    nc.vector.tensor_tensor(
                        out=scT_sb[0:64, pr, ic * 512:(ic + 1) * 512],
                        in0=o_ps[0:64, :],
                        in1=bc_sb[0:64, 0:512],
                        op=mybir.AluOpType.mult,
                    )

            # ---- interleaved schedule ----
            # chunk 0 QKV upfront (its x DMAs were interleaved with wqkT)
            for g in range(8):
                qkv_group(xt0, 0, g)
            xts = {0: xt0}
            for ic in range(4):
                pending = []
                if ic < 3:
                    xts[ic + 1] = xt_dma(ic + 1)
                    xt_next = xts[ic + 1]
                    pending += [
                        (lambda g=g, x=xt_next, t=ic + 1: qkv_group(x, t, g))
                        for g in range(8)
                    ]
                if ic > 0:
                    pending += [
                        (lambda u=u, i=ic - 1: proj_unit(i, u))
                        for u in range(8)
                    ]
                attention_ic(ic, pending)
            for u in range(8):
                proj_unit(3, u)

    nc.compile()
    return nc


def _shard_inputs(x, Wqkv, bqkv, Wproj):
    """Build the 8 per-core input maps."""
    x = np.asarray(x, dtype=np.float32)
    Wqkv = np.asarray(Wqkv, dtype=np.float32)
    bqkv = np.asarray(bqkv, dtype=np.float32)
    Wproj = np.asarray(Wproj, dtype=np.float32)

    xTs = [np.ascontiguousarray(x[b].T) for b in range(B)]
    WprojT = np.ascontiguousarray(Wproj.T)  # [D, D]; rows = concat dim c

    in_maps = []
    for c in range(NCORES):
        b, hg = divmod(c, HG)
        r0 = 256 * hg
        wq = Wqkv[r0:r0 + 256]            # [256, D] heads 4hg..4hg+3 (q)
        wk = Wqkv[D + r0:D + r0 + 256]    # (k)
        wv = Wqkv[2 * D + r0:2 * D + r0 + 256]  # (v)
        wqkT = np.ascontiguousarray(np.concatenate([wq, wk], axis=0).T)
        wvT = np.ascontiguousarray(wv.T)
        bq = bqkv[r0:r0 + 256]
        bk = bqkv[D + r0:D + r0 + 256]
        bv = np.ascontiguousarray(bqkv[2 * D + r0:2 * D + r0 + 256])
        bqk = np.ascontiguousarray(
            np.concatenate([bq, bk]).reshape(4, 128).T
        )  # [128, 4]: cols = q pair0, q pair1, k pair0, k pair1
        wpT = np.ascontiguousarray(WprojT[r0:r0 + 256, :])  # [256, D]
        in_maps.append(
            {
                "xT": xTs[b],
                "wqkT": wqkT,
                "wvT": wvT,
                "bqk": bqk,
                "bv": bv,
                "wpT": wpT,
            }
        )
    return in_maps


def kernel(x, Wqkv, bqkv, Wproj, bproj):
    global _cached_nc, LAST_RESULTS
    if _cached_nc is None:
        _cached_nc = _build_nc()
    nc = _cached_nc

    in_maps = _shard_inputs(x, Wqkv, bqkv, Wproj)
    res = run_bass_kernel_spmd(
        nc, in_maps, core_ids=list(range(NCORES)), trace=TRACE
    )
    LAST_RESULTS = res

    bproj = np.asarray(bproj, dtype=np.float32)
    parts = [res.results[c]["out"] for c in range(NCORES)]
    out = np.stack(
        [
            parts[b * HG]
            + parts[b * HG + 1]
            + parts[b * HG + 2]
            + parts[b * HG + 3]
            + bproj[None, :]
            for b in range(B)
        ]
    )
    return out.astype(np.float32)



# revision 17
# speedup vs baseline: 1.2197x; 1.2197x over previous
"""Causal MHA (B=2, T=2048, D=1024, H=16, HD=64) on 8 TRN2 NeuronCores.

Sharding (Megatron-style, per the hint): core c = b*4 + hg handles batch b
and head group hg (4 heads). Each core computes
  qkv shard -> 4-head causal attention -> partial proj output [T, D]
and the host reduces the 4 per-batch partials and adds bproj.

Device dataflow (zero on-device DMA transposes; host pre-transposes x/weights):
  xT [D, T] (host: x[b].T)
  qT,kT = Wqk_shard @ x  computed as matmul(lhsT=wqkT, rhs=xT) -> [r, t] layout
  v     = x @ Wv_shard.T computed as matmul(lhsT=xT, rhs=wvT)  -> [t, r] layout
  S^T[j, i] = matmul(lhsT=kT, rhs=qT)  (K=64 contraction, 2 heads of the pair
              via base_partition 0/64 into one [P, 2, 512] PSUM tile)
  es = exp(0.125 * S^T) in bf16  (no max-subtraction: logits are O(1))
  causal handled structurally; diagonal 128x128 sub-blocks masked
          multiplicatively post-exp (bf16 mask, 4x DVE mode)
  AV flipped vs the usual: es (bf16) is the STATIONARY operand, va [j, 65]
          (v row-block + ones column) is the MOVING operand, so each matmul
          streams only N=65 columns -> scores land NATURALLY as [i, m] with
          the softmax denominator in column 64/129 (ones column of va).
          Cost model charges ap_size(out)=65 per matmul vs 512 for the
          scores^T orientation: ~2x fewer PE cycles for AV.
  normalize = per-partition reciprocal + tensor_scalar_mul (den is a column
          in the natural layout) -> packed [t, 128] bf16 pair-stacked scores
  scores^T for proj via PE transpose (bf16, 128 cyc) into the dead AV psum
          slot, then strided DVE copies -> scT sbuf
  out = matmul(lhsT=scores^T, rhs=wprojT) -> [t, j] natural layout

Scheduling: QKV t-chunks and attention i-chunks interleave causally; AV for
j-block jt is emitted one iteration late (lag-1 software pipeline) so exp(jt)
on ACT hides behind S(jt+1)+AV(jt) on PE. Next-chunk QKV groups and
prev-chunk proj units are spread across jt iterations as PE filler.

QKV/S/proj matmuls are float32r (TF32-like, 1 cyc/row at N>=256);
the AV/transpose path is bf16 (1 cyc/row at any N). PSUM accumulates fp32.
PSUM budget (8 banks): qkv/proj 2, S^T 4 (double-buffered [P,1024]),
AV accum 2 ([P,512] x2, two 130-col slots each; transposes reuse dead slots).
"""

import numpy as np

import concourse.mybir as mybir
import concourse.tile as tile
from concourse import bacc
from concourse.bass_utils import run_bass_kernel_spmd
from concourse.masks import make_upper_triangular, make_identity

B, T, D, H = 2, 2048, 1024, 16
HD = 64
NCORES = 8
HG = 4          # head groups (cores per batch)
HPG = H // HG   # heads per core = 4
P = 128
F32 = mybir.dt.float32
F32R = mybir.dt.float32r
BF16 = mybir.dt.bfloat16
EXP = mybir.ActivationFunctionType.Exp

# module-level knobs for test harness
TRACE = False
LAST_RESULTS = None
DEBUG_DUMPS = False

_cached_nc = None


def _build_nc():
    nc = bacc.Bacc("TRN2", target_bir_lowering=False, debug=False)

    xT_d = nc.dram_tensor("xT", [D, T], F32R, kind="ExternalInput")
    wqkT_d = nc.dram_tensor("wqkT", [D, 512], F32R, kind="ExternalInput")
    wvT_d = nc.dram_tensor("wvT", [D, 256], F32R, kind="ExternalInput")
    bqk_d = nc.dram_tensor("bqk", [P, 4], F32, kind="ExternalInput")
    bv_d = nc.dram_tensor("bv", [256], F32, kind="ExternalInput")
    wpT_d = nc.dram_tensor("wpT", [256, D], F32R, kind="ExternalInput")
    out_d = nc.dram_tensor("out", [T, D], F32, kind="ExternalOutput")
    if DEBUG_DUMPS:
        scT_dump = nc.dram_tensor("scT_dump", [P, 2 * T], F32R, kind="ExternalOutput")
        qT_dump = nc.dram_tensor("qT_dump", [P, 2 * T], F32R, kind="ExternalOutput")
        kT_dump = nc.dram_tensor("kT_dump", [P, 2 * T], F32R, kind="ExternalOutput")
        va_dump = nc.dram_tensor("va_dump", [P, 2 * 16 * 2 * 65], F32, kind="ExternalOutput")
        es_dump = nc.dram_tensor("es_dump", [P, 4 * 1024], F32, kind="ExternalOutput")
        av_dump = nc.dram_tensor("av_dump", [P, 4 * 130], F32, kind="ExternalOutput")
        pk_dump = nc.dram_tensor("pk_dump", [P, 4 * 128], F32, kind="ExternalOutput")

    import concourse.bass as bass

    with tile.TileContext(nc) as tc:
        with (
            tc.tile_pool(name="consts", bufs=1) as consts,
            tc.tile_pool(name="work", bufs=2) as work,
            tc.tile_pool(name="es", bufs=6) as es_pool,
            tc.tile_pool(name="norm", bufs=4) as norm,
            tc.tile_pool(name="outp", bufs=4) as outp,
            tc.tile_pool(name="psA", bufs=2, space="PSUM") as psA,
            tc.tile_pool(name="psQ", bufs=2, space="PSUM") as psQ,
            tc.tile_pool(name="psV", bufs=2, space="PSUM") as psV,
        ):
            # ---- persistent tiles (loads emitted in consumption order) ----
            wqkT_sb = consts.tile([P, 8, 512], F32R)
            wqkT_r = wqkT_d[:].rearrange("(ko p) r -> p ko r", p=P)
            wvT_sb = consts.tile([P, 8, 256], F32R)
            wpT_sb = consts.tile([P, 2, D], F32R)
            bqk_sb = consts.tile([P, 4], F32)
            nc.sync.dma_start(bqk_sb[:], bqk_d[:])
            bv_sb = consts.tile([P, 256], F32)
            nc.sync.dma_start(
                bv_sb[:],
                bass.AP(tensor=bv_d, offset=0, ap=[[0, P], [1, 256]]),
            )
            # masks / identity in bf16 (built via f32 then converted)
            mask_f = consts.tile([P, P], F32)
            make_upper_triangular(nc, mask_f[:], val=1.0, diag=True)
            mask_sb = consts.tile([P, P], BF16)
            nc.vector.tensor_copy(mask_sb[:], mask_f[:])
            idn_f = consts.tile([P, P], F32)
            make_identity(nc, idn_f[:])

            qT_sb = consts.tile([P, 2, T], F32R)
            kT_sb = consts.tile([P, 2, T], F32R)
            # va[j, jt, pr, 0:65]: per j-block, per pair: head's v rows in
            # bf16 + ones column 64 (AV's moving operand; the ones column
            # accumulates the softmax denominator for free).
            vaA_sb = consts.tile([P, 16, 2, 65], BF16)
            vaB_sb = consts.tile([P, 16, 2, 65], BF16)
            nc.vector.memset(vaA_sb[:, :, :, 64:65], 1.0)
            nc.vector.memset(vaB_sb[:, :, :, 64:65], 1.0)
            scT_sb = consts.tile([P, 2, T], F32R)

            xT_r = xT_d[:].rearrange("(ko p) t -> p ko t", p=P)

            # chunk-0 x interleaved with wqkT so the first q/k matmul can
            # start after ~512KB of traffic; bulkier weights follow.
            xt0 = work.tile([P, 8, 512], F32R, tag="xt")
            for ko in range(8):
                nc.sync.dma_start(xt0[:, ko, :], xT_r[:, ko, 0:512])
                nc.sync.dma_start(wqkT_sb[:, ko, :], wqkT_r[:, ko, :])
            nc.sync.dma_start(
                wvT_sb[:], wvT_d[:].rearrange("(ko p) r -> p ko r", p=P)
            )
            nc.sync.dma_start(
                wpT_sb[:], wpT_d[:].rearrange("(co p) j -> p co j", p=P)
            )

            # ---- work-unit emitters ----
            def xt_dma(tch):
                xt = work.tile([P, 8, 512], F32R, tag="xt")
                for ko in range(8):
                    nc.sync.dma_start(
                        xt[:, ko, :],
                        xT_r[:, ko, tch * 512:(tch + 1) * 512],
                    )
                return xt

            def qkv_group(xt, tch, g):
                """g 0..3: q/k r-tiles; g 4..7: v t-tiles."""
                if g < 4:
                    rt = g
                    ps = psQ.tile([P, 512], F32, tag="qkv")
                    for ko in range(8):
                        nc.tensor.matmul(
                            ps[:],
                            wqkT_sb[:, ko, rt * 128:(rt + 1) * 128],
                            xt[:, ko, :],
                            start=(ko == 0),
                            stop=(ko == 7),
                        )
                    dest = (qT_sb if rt < 2 else kT_sb)[
                        :, rt % 2, tch * 512:(tch + 1) * 512
                    ]
                    nc.vector.tensor_scalar_add(
                        out=dest, in0=ps[:], scalar1=bqk_sb[:, rt:rt + 1]
                    )
                else:
                    tt4 = g - 4
                    tg = tch * 4 + tt4
                    psv = psQ.tile([P, 256], F32, tag="qkv")
                    for ko in range(8):
                        nc.tensor.matmul(
                            psv[:],
                            xt[:, ko, tt4 * 128:(tt4 + 1) * 128],
                            wvT_sb[:, ko, :],
                            start=(ko == 0),
                            stop=(ko == 7),
                        )
                    psv_h = psv[:].rearrange("p (pr h e) -> p pr h e", pr=2, h=2)
                    bv_h = bv_sb[:].rearrange("p (pr h e) -> p pr h e", pr=2, h=2)
                    nc.vector.tensor_tensor(
                        out=vaA_sb[:, tg, :, 0:64],
                        in0=psv_h[:, :, 0, :],
                        in1=bv_h[:, :, 0, :],
                        op=mybir.AluOpType.add,
                    )
                    nc.vector.tensor_tensor(
                        out=vaB_sb[:, tg, :, 0:64],
                        in0=psv_h[:, :, 1, :],
                        in1=bv_h[:, :, 1, :],
                        op=mybir.AluOpType.add,
                    )

            def proj_unit(ic, u):
                """u 0..7: tt4 = u // 2, jc = u % 2."""
                tt = ic * 4 + u // 2
                jc = u % 2
                pp = psQ.tile([P, 512], F32, tag="qkv")
                for pr in range(2):
                    nc.tensor.matmul(
                        pp[:],
                        scT_sb[:, pr, tt * 128:(tt + 1) * 128],
                        wpT_sb[:, pr, jc * 512:(jc + 1) * 512],
                        start=(pr == 0),
                        stop=(pr == 1),
                    )
                o_sb = outp.tile([P, 512], F32, tag="out")
                nc.vector.tensor_copy(o_sb[:], pp[:])
                nc.sync.dma_start(
                    out_d[tt * 128:(tt + 1) * 128, jc * 512:(jc + 1) * 512],
                    o_sb[:],
                )

            def attention_ic(ic, pending):
                """pending: list of zero-arg emitters interleaved between
                jt iterations (PE filler)."""
                njt = 4 * ic + 4
                niter = 2 * njt
                k = len(pending)
                sched = {}
                for idx in range(k):
                    sched.setdefault(idx * niter // k, []).append(pending[idx])
                it = 0
                for pr in range(2):
                    # AV accumulators: 2 banks x 2 slots (cols 0 / 256),
                    # slot itl holds scores+den for i-tile ic*4+itl:
                    # head A cols off..off+65, head B off+65..off+130.
                    # PSUM start=True zeroes a whole 2KB region lazily, so
                    # independent col-disjoint groups in one bank cannot
                    # each use start: memset the bank once and accumulate
                    # with start=False onto the zeros instead.
                    avX = psV.tile([P, 512], F32, tag="av")
                    avY = psV.tile([P, 512], F32, tag="av")
                    nc.vector.memset(avX[:], 0.0)
                    nc.vector.memset(avY[:], 0.0)

                    def av_slot(itl):
                        return (avX if itl < 2 else avY), 256 * (itl % 2)

                    es_tiles = {}
                    pend_tr = []

                    def emit_av(jt):
                        """AV matmuls for j-block jt + normalize of the
                        i-tile whose diagonal is jt (lag-1 pipelined); the
                        PE transpose is deferred one more iteration so the
                        DVE normalize chain hides behind the next S+AV."""
                        if pend_tr:
                            pend_tr.pop()()
                        m = jt - 4 * ic
                        es = es_tiles.pop(jt)
                        for itl in range(max(0, m), 4):
                            av, off = av_slot(itl)
                            for h, va in ((0, vaA_sb), (1, vaB_sb)):
                                nc.tensor.matmul(
                                    av[:, off + 65 * h:off + 65 * h + 65],
                                    es[:, h, itl * 128:(itl + 1) * 128],
                                    va[:, jt, pr, :],
                                    start=False,
                                    stop=(jt == 4 * ic + itl),
                                    skip_group_check=True,
                                )
                        if m >= 0:
                            # i-tile itl=m is complete: normalize (den at
                            # cols off+64 / off+129), pack bf16; transpose
                            # into the now-dead slot region next iteration.
                            av, off = av_slot(m)
                            if DEBUG_DUMPS and pr == 0 and ic == 0:
                                dsb = consts.tile(
                                    [P, 130], F32, name=f"avd{m}"
                                )
                                nc.vector.tensor_copy(
                                    dsb[:], av[:, off:off + 130]
                                )
                                nc.sync.dma_start(
                                    av_dump[:, m * 130:(m + 1) * 130], dsb[:]
                                )
                            rec = norm.tile([P, 2], F32, tag="rec")
                            nc.vector.reciprocal_approx_fast(
                                out=rec[:], in_=av[:, off + 64:off + 130:65]
                            )
                            packed = norm.tile([P, P], F32, tag="packed")
                            nc.vector.tensor_scalar_mul(
                                out=packed[:, 0:64],
                                in0=av[:, off:off + 64],
                                scalar1=rec[:, 0:1],
                            )
                            nc.vector.tensor_scalar_mul(
                                out=packed[:, 64:128],
                                in0=av[:, off + 65:off + 129],
                                scalar1=rec[:, 1:2],
                            )

                            if DEBUG_DUMPS and pr == 0 and ic == 0:
                                pksb = consts.tile(
                                    [P, 128], F32, name=f"pkd{m}"
                                )
                                nc.vector.tensor_copy(pksb[:], packed[:])
                                nc.sync.dma_start(
                                    pk_dump[:, m * 128:(m + 1) * 128],
                                    pksb[:],
                                )

                            def transpose(packed=packed, itl=m):
                                trt = psQ.tile([P, P], F32, tag="qkv")
                                nc.tensor.matmul(
                                    trt[:],
                                    packed[:],
                                    idn_f[:],
                                    start=True,
                                    stop=True,
                                    is_transpose=True,
                                )
                                nc.vector.tensor_copy(
                                    scT_sb[
                                        :, pr,
                                        ic * 512 + itl * 128:
                                        ic * 512 + (itl + 1) * 128,
                                    ],
                                    trt[:],
                                )

                            pend_tr.append(transpose)

                    for jt in range(njt):
                        m = jt - 4 * ic
                        # S granularity: widen the m==3 tile to N=256 to
                        # dodge the f32r narrow-matmul penalty; exp/mask
                        # only cover the true range 128m..512.
                        s_lo = 0 if m <= 0 else (256 if m == 3 else 128 * m)
                        e_lo = 0 if m <= 0 else 128 * m
                        s_ps = psA.tile([P, 1024], F32, tag="s")
                        for h in range(2):
                            nc.tensor.matmul(
                                s_ps[:, h * 512 + s_lo:(h + 1) * 512],
                                kT_sb[64 * h:64 * h + 64, pr,
                                      jt * 128:(jt + 1) * 128],
                                qT_sb[64 * h:64 * h + 64, pr,
                                      ic * 512 + s_lo:(ic + 1) * 512],
                                start=True,
                                stop=True,
                            )
                        es = es_pool.tile([P, 2, 512], BF16, tag="es")
                        es_tiles[jt] = es
                        s_view = s_ps[:].rearrange("p (h i) -> p h i", h=2)
                        nc.scalar.activation(
                            out=es[:, :, e_lo:],
                            in_=s_view[:, :, e_lo:],
                            func=EXP,
                            scale=0.125,
                        )
                        if m >= 0:
                            nc.vector.tensor_tensor(
                                out=es[:, :, e_lo:e_lo + 128],
                                in0=es[:, :, e_lo:e_lo + 128],
                                in1=mask_sb[:, None, :].to_broadcast([P, 2, P]),
                                op=mybir.AluOpType.mult,
                            )
                        if DEBUG_DUMPS and pr == 0 and ic == 0:
                            esb = consts.tile([P, 2, 512], F32, name=f"esd{jt}")
                            nc.vector.tensor_copy(esb[:], es[:])
                            nc.sync.dma_start(
                                es_dump[:, jt * 1024:(jt + 1) * 1024].rearrange(
                                    "p (h i) -> p h i", h=2
                                ),
                                esb[:],
                            )
                        if jt > 0:
                            emit_av(jt - 1)
                        for fill in sched.get(it, ()):
                            fill()
                        it += 1
                    emit_av(njt - 1)
                    while pend_tr:
                        pend_tr.pop()()

            # ---- interleaved schedule ----
            # chunk 0 QKV upfront (its x DMAs were interleaved with wqkT)
            for g in range(8):
                qkv_group(xt0, 0, g)
            xts = {0: xt0}
            for ic in range(4):
                pending = []
                if ic < 3:
                    xts[ic + 1] = xt_dma(ic + 1)
                    xt_next = xts[ic + 1]
                    pending += [
                        (lambda g=g, x=xt_next, t=ic + 1: qkv_group(x, t, g))
                        for g in range(8)
                    ]
                if ic > 0:
                    pending += [
                        (lambda u=u, i=ic - 1: proj_unit(i, u))
                        for u in range(8)
                    ]
                attention_ic(ic, pending)
            for u in range(8):
                proj_unit(3, u)

            if DEBUG_DUMPS:
                nc.sync.dma_start(
                    scT_dump[:].rearrange("p (a t) -> p a t", a=2), scT_sb[:]
                )
                nc.sync.dma_start(
                    qT_dump[:].rearrange("p (a t) -> p a t", a=2), qT_sb[:]
                )
                nc.sync.dma_start(
                    kT_dump[:].rearrange("p (a t) -> p a t", a=2), kT_sb[:]
                )
                va_f = consts.tile([P, 2, 16, 2, 65], F32)
                nc.vector.tensor_copy(va_f[:, 0], vaA_sb[:])
                nc.vector.tensor_copy(va_f[:, 1], vaB_sb[:])
                nc.sync.dma_start(
                    va_dump[:].rearrange(
                        "p (a j r e) -> p a j r e", a=2, j=16, r=2
                    ),
                    va_f[:],
                )

    nc.compile()
    return nc


def _shard_inputs(x, Wqkv, bqkv, Wproj):
    """Build the 8 per-core input maps."""
    x = np.asarray(x, dtype=np.float32)
    Wqkv = np.asarray(Wqkv, dtype=np.float32)
    bqkv = np.asarray(bqkv, dtype=np.float32)
    Wproj = np.asarray(Wproj, dtype=np.float32)

    xTs = [np.ascontiguousarray(x[b].T) for b in range(B)]
    WprojT = np.ascontiguousarray(Wproj.T)  # [D, D]; rows = concat dim c

    in_maps = []
    for c in range(NCORES):
        b, hg = divmod(c, HG)
        r0 = 256 * hg
        wq = Wqkv[r0:r0 + 256]            # [256, D] heads 4hg..4hg+3 (q)
        wk = Wqkv[D + r0:D + r0 + 256]    # (k)
        wv = Wqkv[2 * D + r0:2 * D + r0 + 256]  # (v)
        wqkT = np.ascontiguousarray(np.concatenate([wq, wk], axis=0).T)
        wvT = np.ascontiguousarray(wv.T)
        bq = bqkv[r0:r0 + 256]
        bk = bqkv[D + r0:D + r0 + 256]
        bv = np.ascontiguousarray(bqkv[2 * D + r0:2 * D + r0 + 256])
        bqk = np.ascontiguousarray(
            np.concatenate([bq, bk]).reshape(4, 128).T
        )  # [128, 4]: cols = q pair0, q pair1, k pair0, k pair1
        wpT = np.ascontiguousarray(WprojT[r0:r0 + 256, :])  # [256, D]
        in_maps.append(
            {
                "xT": xTs[b],
                "wqkT": wqkT,
                "wvT": wvT,
                "bqk": bqk,
                "bv": bv,
                "wpT": wpT,
            }
        )
    return in_maps


def kernel(x, Wqkv, bqkv, Wproj, bproj):
    global _cached_nc, LAST_RESULTS
    if _cached_nc is None:
        _cached_nc = _build_nc()
    nc = _cached_nc

    in_maps = _shard_inputs(x, Wqkv, bqkv, Wproj)
    res = run_bass_kernel_spmd(
        nc, in_maps, core_ids=list(range(NCORES)), trace=TRACE
    )
    LAST_RESULTS = res

    bproj = np.asarray(bproj, dtype=np.float32)
    parts = [res.results[c]["out"] for c in range(NCORES)]
    out = np.stack(
        [
            parts[b * HG]
            + parts[b * HG + 1]
            + parts[b * HG + 2]
            + parts[b * HG + 3]
            + bproj[None, :]
            for b in range(B)
        ]
    )
    return out.astype(np.float32)
